# revision 1
# baseline (speedup 1.0000x reference)
"""Trainium2 Bass kernel for nn_CrossAtt (dual cross-attention + 3x3 conv + BN + ReLU).

Sharding: 8 cores = (sample s in 0..3) x (h-half in 0..1). Each core computes
its 32 output rows plus a 1-row attention halo on each side (34 rows = 2176
query positions, host-zero-padded so the program is SPMD-uniform), then runs
the 3x3 conv locally. No collectives.

Device layout choices:
- scoresT [m, n] comes straight off the PE (lhsT=k zero-padded to K=128,
  rhs=q), so softmax needs no transpose of the 4096x2176 matrix.
- exp on ScalarE (no max subtraction; |scores| <~ 5 so fp32 exp is safe).
- AV: out^T[n, 257] = expT.T @ [vT | ones]; col 256 accumulates the softmax
  denominator S for free.
- normalize by (gamma * mask / S) as a per-partition scalar; mask zeroes the
  fake padded query rows. PE-transposes the small [n,256] result to [256,n].
- residual + gamma*bv bias are folded into the host-prepared x?r inputs.
- conv3x3 = 9 shifted matmuls over a [512, 35*66] zero-padded cat buffer;
  BN+ReLU fused into one activation (scale=inv, bias=beta per partition).
"""
import sys

if "/opt/trn_rl_repo" not in sys.path:
    sys.path.insert(0, "/opt/trn_rl_repo")

import numpy as np

import concourse.bass as bass
import concourse.bacc as bacc
import concourse.mybir as mybir
import concourse.tile as tile
from concourse.bass import ds, ts
from concourse.bass_utils import run_bass_kernel_spmd

F32 = mybir.dt.float32
F32R = mybir.dt.float32r  # same bits as fp32; 1 cycle/row PE mode (vs 4 for fp32)
EPS = 1e-5
P = 128
C = 256          # channels
M = 4096         # key/value positions (64*64)
NQ = 2176        # query positions per core (34 rows * 64), host padded
NROWS = 35       # cat_pad rows (34 data + 1 zero)
WPAD = 66        # cat_pad row width (64 + 2 zero cols)
# all moving dims >= 256 so the fp32r fast path applies
ATT_BLOCKS = [(i * 256, 256) for i in range(8)] + [(2048, 128)]
QWINS = [(0, 512), (512, 512), (1024, 512), (1536, 384), (1920, 256)]
CONV_WINS = [(1, 512), (513, 512), (1025, 512), (1537, 318), (1855, 256)]

_CACHE = {}


def _wins(total, w):
    return [(i, min(w, total - i)) for i in range(0, total, w)]


def _mm(nc, out, lhsT, rhs, **kw):
    nc.tensor.matmul(out, lhsT, rhs, **kw)


def _declare_io(nc):
    t = {}
    inp = lambda name, shape, dt=F32: t.__setitem__(
        name, nc.dram_tensor(name, shape, dt, kind="ExternalInput"))
    out = lambda name, shape, dt=F32: t.__setitem__(
        name, nc.dram_tensor(name, shape, dt, kind="ExternalOutput"))
    # fp32r = same 32-bit data; matmul operands must be declared fp32r end-to-end
    inp("x1", [C, M], F32R); inp("x2", [C, M], F32R)
    inp("x1q", [C, NQ], F32R); inp("x2q", [C, NQ], F32R)
    inp("x1r", [C, NQ]); inp("x2r", [C, NQ])
    inp("maskg", [P, 17])
    inp("wq1T", [P, 2, 32], F32R); inp("wq2T", [P, 2, 32], F32R)
    inp("wk1T", [P, 2, 64], F32R); inp("wk2T", [P, 2, 64], F32R)
    inp("wv1T", [P, 2, C], F32R); inp("wv2T", [P, 2, C], F32R)
    inp("bq1", [32, 1]); inp("bq2", [32, 1])
    inp("bk1", [64, 1]); inp("bk2", [64, 1])
    inp("cinv", [P, 2]); inp("cbeta", [P, 2])
    inp("wct", [3, 3, 2 * C, C], F32R)
    inp("ident", [P, P])
    out("feat", [C, 32, 64]); out("o1", [C, 32, 64], F32R); out("o2", [C, 32, 64], F32R)
    return t


def _emit(nc, tc, t, ctx):
    big = ctx.enter_context(tc.tile_pool(name="big", bufs=3))
    kqp = ctx.enter_context(tc.tile_pool(name="kq", bufs=1))
    sing = ctx.enter_context(tc.tile_pool(name="sing", bufs=1))
    expp = ctx.enter_context(tc.tile_pool(name="expp", bufs=3))
    normp = ctx.enter_context(tc.tile_pool(name="normp", bufs=3))
    scalp = ctx.enter_context(tc.tile_pool(name="scalp", bufs=4))
    resp = ctx.enter_context(tc.tile_pool(name="resp", bufs=4))
    wcp = ctx.enter_context(tc.tile_pool(name="wcp", bufs=12))
    psA = ctx.enter_context(tc.tile_pool(name="psA", bufs=2, space="PSUM"))
    psS = ctx.enter_context(tc.tile_pool(name="psS", bufs=2, space="PSUM"))

    BIG_SHAPE_BYTES = [P, 4 * NROWS * WPAD]  # cat_pad is the largest big tile

    # ---- constants / weights to SBUF ----
    idt = sing.tile([P, P], F32)
    nc.sync.dma_start(out=idt, in_=t["ident"][:])
    wq_sb, wk_sb, wv_sb, bq_sb, bk_sb = {}, {}, {}, {}, {}
    for b in (1, 2):
        wq_sb[b] = sing.tile([P, 2, 32], F32R, tag=f"wq{b}", name=f"wq{b}")
        nc.sync.dma_start(out=wq_sb[b], in_=t[f"wq{b}T"][:])
        wk_sb[b] = sing.tile([P, 2, 64], F32R, tag=f"wk{b}", name=f"wk{b}")
        nc.sync.dma_start(out=wk_sb[b], in_=t[f"wk{b}T"][:])
        wv_sb[b] = sing.tile([P, 2, C], F32R, tag=f"wv{b}", name=f"wv{b}")
        nc.sync.dma_start(out=wv_sb[b], in_=t[f"wv{b}T"][:])
        bq_sb[b] = sing.tile([32, 1], F32, tag=f"bq{b}", name=f"bq{b}")
        nc.sync.dma_start(out=bq_sb[b], in_=t[f"bq{b}"][:])
        bk_sb[b] = sing.tile([64, 1], F32, tag=f"bk{b}", name=f"bk{b}")
        nc.sync.dma_start(out=bk_sb[b], in_=t[f"bk{b}"][:])
    cinv_sb = sing.tile([P, 2], F32, tag="cinv")
    nc.sync.dma_start(out=cinv_sb, in_=t["cinv"][:])
    cbeta_sb = sing.tile([P, 2], F32, tag="cbeta")
    nc.sync.dma_start(out=cbeta_sb, in_=t["cbeta"][:])
    maskg_sb = sing.tile([P, 17], F32, tag="maskg")
    nc.sync.dma_start(out=maskg_sb, in_=t["maskg"][:])

    # ---- load x1, x2 (two column-half DMAs so the PE can start earlier) ----
    def load_x(name):
        x_sb = big.tile(BIG_SHAPE_BYTES, F32R, tag="big")
        xv = x_sb[:, : 2 * M].rearrange("p (kc n) -> p kc n", kc=2)
        src_ap = t[name][:].rearrange("(kc p) n -> p kc n", p=P)
        for c0 in range(0, M, 1024):
            nc.sync.dma_start(out=xv[:, :, ds(c0, 1024)],
                              in_=src_ap[:, :, ds(c0, 1024)])
        return xv

    x1_sb = load_x("x1")
    x2_sb = load_x("x2")

    # ---- k projections: k_b = wk_b @ x_b + bk_b, stored [128(c pad0), 4096] ----
    k_sb = {}
    for b, x_sb in ((1, x1_sb), (2, x2_sb)):
        kp = kqp.tile([P, M], F32R, tag=f"k{b}")
        for w0, ww in _wins(M, 512):
            ps = psS.tile([P, 1024], F32, tag="sc")
            for kc in range(2):
                _mm(nc, ps[0:64, :ww], wk_sb[b][:, kc, :],
                    x_sb[:, kc, ds(w0, ww)],
                    start=(kc == 0), stop=(kc == 1))
            nc.vector.tensor_scalar_add(kp[0:64, ds(w0, ww)], ps[0:64, :ww], bk_sb[b])
        k_sb[b] = kp

    # ---- vT projections: vT_b[m, c] = x_b.T @ wv_bT (no bias), plus ones col ----
    def make_vt(x_sb, b):
        vt = big.tile(BIG_SHAPE_BYTES, F32R, tag="big")
        vtv = vt[:, : 32 * 258].rearrange("p (mi c) -> p mi c", mi=32)
        nc.vector.memset(vtv[:, :, 256:257].bitcast(F32), 1.0)
        nc.vector.memset(vtv[:, :, 257:258].bitcast(F32), 0.0)
        for mi in range(32):
            ps_full = psS.tile([P, 1024], F32, tag="sc", name="vtps")
            ps = ps_full[:, :256]
            for kc in range(2):
                _mm(nc, ps, x_sb[:, kc, ts(mi, P)], wv_sb[b][:, kc, :],
                    start=(kc == 0), stop=(kc == 1))
            nc.vector.tensor_copy(out=vtv[:, mi, 0:256], in_=ps)
        return vtv

    # ---- q projection (shared by both branches): qp [128(c pad0), 2176] ----
    qp = kqp.tile([P, NQ], F32R, tag="qp")

    def q_half(name, b, row0):
        xq = big.tile(BIG_SHAPE_BYTES, F32R, tag="big")
        xqv = xq[:, : 2 * NQ].rearrange("p (kc n) -> p kc n", kc=2)
        xq_src = t[name][:].rearrange("(kc p) n -> p kc n", p=P)
        nc.sync.dma_start(out=xqv[:, :, 0:1088], in_=xq_src[:, :, 0:1088])
        nc.sync.dma_start(out=xqv[:, :, 1088:NQ], in_=xq_src[:, :, 1088:NQ])
        for w0, ww in QWINS:
            ps = psS.tile([P, 1024], F32, tag="sc")
            for kc in range(2):
                _mm(nc, ps[0:32, :ww], wq_sb[b][:, kc, :],
                    xqv[:, kc, ds(w0, ww)],
                    start=(kc == 0), stop=(kc == 1))
            nc.vector.tensor_scalar_add(qp[row0:row0 + 32, ds(w0, ww)],
                                        ps[0:32, :ww], bq_sb[b])

    q_half("x1q", 1, 0)
    vt1 = make_vt(x1_sb, 1)
    q_half("x2q", 2, 32)
    vt2 = make_vt(x2_sb, 2)

    # ---- cat_pad buffer [128, 4, 35*66], zeroed ----
    cat = big.tile(BIG_SHAPE_BYTES, F32R, tag="big")
    catv = cat[:].rearrange("p (i f) -> p i f", i=4)
    cat_r = cat[:].rearrange("p (i r w) -> p i r w", i=4, w=WPAD)
    nc.gpsimd.memset(cat[:].bitcast(F32), 0.0)

    # ---- attention branches ----
    for b, (kp, vtv, xr_name) in enumerate(
            [(k_sb[1], vt1, "x1r"), (k_sb[2], vt2, "x2r")]):
        for n0, nw in ATT_BLOCKS:
            nsub = nw // P
            g = 1024 // nw  # m-iters per exp group (4 for nw=256, 8 for 128)
            av = psA.tile([P, 1024], F32, tag="av")

            def flush_av(pend, av=av, vtv=vtv, nw=nw, nsub=nsub):
                g0, ex = pend
                for u in range(1024 // nw):
                    pmi = g0 + u
                    for j in range(nsub):
                        _mm(nc, av[:, ds(j * 512, 258)],
                            ex[:, ds(u * nw + j * P, P)], vtv[:, pmi, :],
                            start=(pmi == 0), stop=(pmi == 31))

            pend = None
            for g0 in range(0, 32, g):
                sc = psS.tile([P, 1024], F32, tag="sc")
                for u in range(g):
                    mi = g0 + u
                    _mm(nc, sc[:, ds(u * nw, nw)],
                        kp[0:64, ts(mi, P)], qp[0:64, ds(n0, nw)],
                        start=True, stop=True)
                ex = expp.tile([P, 1024], F32R, tag="ex")
                nc.scalar.activation(ex, sc, mybir.ActivationFunctionType.Exp)
                if pend is not None:
                    flush_av(pend)
                pend = (g0, ex)
            flush_av(pend)

            # epilogue per n-chunk of 128; transposes reuse the consumed AV bank
            for j in range(nsub):
                nch = n0 // P + j
                rs = scalp.tile([P, 1], F32, tag="rs")
                nc.vector.reciprocal(rs, av[:, ds(j * 512 + 256, 1)])
                nc.vector.tensor_mul(out=rs, in0=rs,
                                     in1=maskg_sb[:, ds(nch, 1)])
                nt = normp.tile([P, 256], F32, tag="nt")
                nc.vector.tensor_scalar_mul(nt, av[:, ds(j * 512, 256)], rs)
                rt = resp.tile([P, 2, P], F32, tag="rt")
                nc.sync.dma_start(
                    out=rt,
                    in_=t[xr_name][:].rearrange("(cc p) n -> p cc n", p=P)
                    [:, :, ts(nch, P)])
                for cc in range(2):
                    tp = av[:, ds(j * 512 + cc * P, P)]
                    nc.tensor.transpose(tp, nt[:, ts(cc, P)], idt)
                    nc.vector.tensor_add(
                        out=cat_r[:, 2 * b + cc, ds(2 * nch, 2), ds(1, 64)],
                        in0=tp.rearrange("p (r w) -> p r w", w=64),
                        in1=rt[:, cc, :].rearrange("p (r w) -> p r w", w=64))

        # write out this branch's attention output (rows 1..33 = the 32 real rows)
        ov = t[f"o{b + 1}"][:].rearrange("(cc p) h w -> p cc h w", p=P)
        for cc in range(2):
            nc.sync.dma_start(out=ov[:, cc],
                              in_=cat_r[:, 2 * b + cc, ds(1, 32), ds(1, 64)])

    # ---- conv 3x3 + BN + ReLU ----
    feat = big.tile(BIG_SHAPE_BYTES, F32, tag="big")
    featv = feat[:, : 2 * 2112].rearrange("p (o f) -> p o f", o=2)
    feat_r = feat[:, : 2 * 2112].rearrange("p (o r w) -> p o r w", o=2, w=WPAD)
    for oc in range(2):
        avc1 = psA.tile([P, 1024], F32, tag="av")
        avc2 = psA.tile([P, 1024], F32, tag="av")
        last = psS.tile([P, 1024], F32, tag="sc")

        def conv_dst(wi, ww, avc1=avc1, avc2=avc2, last=last):
            if wi < 2:
                return avc1[:, ds(wi * 512, ww)]
            if wi < 4:
                return avc2[:, ds((wi - 2) * 512, ww)]
            return last[:, :ww]

        wts = {}
        for ic in range(4):
            for tap in range(9):
                wt = wcp.tile([P, P], F32R, tag="wt", name=f"wt{oc}_{ic}_{tap}")
                nc.sync.dma_start(
                    out=wt, in_=t["wct"][tap // 3, tap % 3,
                                         ts(ic, P), ts(oc, P)])
                wts[(ic, tap)] = wt
        for ic in range(4):
            for tap in range(9):
                off = (tap // 3) * WPAD + (tap % 3) - 1
                for wi, (ws, ww) in enumerate(CONV_WINS):
                    _mm(nc, conv_dst(wi, ww), wts[(ic, tap)],
                        catv[:, ic, ds(ws + off, ww)],
                        start=(ic == 0 and tap == 0),
                        stop=(ic == 3 and tap == 8))
        for wi, (ws, ww) in enumerate(CONV_WINS):
            nc.scalar.activation(featv[:, oc, ds(ws, ww)], conv_dst(wi, ww),
                                 mybir.ActivationFunctionType.Relu,
                                 bias=cbeta_sb[:, ds(oc, 1)],
                                 scale=cinv_sb[:, ds(oc, 1)])
    fv = t["feat"][:].rearrange("(cc p) h w -> p cc h w", p=P)
    for oc in range(2):
        nc.sync.dma_start(out=fv[:, oc], in_=feat_r[:, oc, :, ds(1, 64)])


def _build():
    if "nc" in _CACHE:
        return _CACHE["nc"]
    nc = bacc.Bacc(None, target_bir_lowering=False)
    t = _declare_io(nc)
    from contextlib import ExitStack
    with tile.TileContext(nc) as tc, ExitStack() as ctx:
        _emit(nc, tc, t, ctx)
    nc.finalize()
    _CACHE["nc"] = nc
    return nc


def _prep_host(inputs):
    d = {k: np.ascontiguousarray(np.asarray(v, np.float32)) for k, v in inputs.items()}
    gamma = float(d["gamma"].reshape(-1)[0])
    inv = d["bn_scale"] / np.sqrt(d["bn_var"] + EPS)
    beta = d["bn_bias"] - d["bn_mean"] * inv

    def chunked(w):  # [256, o] -> [128, 2, o]
        return np.ascontiguousarray(w.reshape(2, P, -1).transpose(1, 0, 2))

    shared = {
        "wq1T": chunked(d["wq1"].T), "wq2T": chunked(d["wq2"].T),
        "wk1T": chunked(d["wk1"].T), "wk2T": chunked(d["wk2"].T),
        "wv1T": chunked(d["wv1"].T), "wv2T": chunked(d["wv2"].T),
        "bq1": d["bq1"].reshape(32, 1).copy(), "bq2": d["bq2"].reshape(32, 1).copy(),
        "bk1": d["bk1"].reshape(64, 1).copy(), "bk2": d["bk2"].reshape(64, 1).copy(),
        "cinv": np.ascontiguousarray(inv.reshape(2, P).T),
        "cbeta": np.ascontiguousarray(beta.reshape(2, P).T),
        "wct": np.ascontiguousarray(d["w_cat"].transpose(2, 3, 1, 0)),
        "ident": np.eye(P, dtype=np.float32),
    }
    gbv = {1: gamma * d["bv1"], 2: gamma * d["bv2"]}

    in_maps = []
    for core in range(8):
        s, half = core // 2, core % 2
        h0 = 32 * half
        x1 = np.ascontiguousarray(d["input1"][s].reshape(C, M))
        x2 = np.ascontiguousarray(d["input2"][s].reshape(C, M))
        n_lo, n_hi = (h0 - 1) * 64, (h0 + 33) * 64
        lo_pad, hi_pad = max(0, -n_lo), max(0, n_hi - M)
        sl = slice(n_lo + lo_pad, n_hi - hi_pad)

        def pad_slice(x, add=None):
            o = np.zeros((C, NQ), np.float32)
            body = x[:, sl]
            if add is not None:
                body = body + add[:, None]
            o[:, lo_pad:NQ - hi_pad] = body
            return o

        maskg = np.zeros(NQ, np.float32)
        maskg[lo_pad:NQ - hi_pad] = gamma
        m = dict(shared)
        m.update({
            "x1": x1, "x2": x2,
            "x1q": pad_slice(x1), "x2q": pad_slice(x2),
            "x1r": pad_slice(x1, gbv[1]), "x2r": pad_slice(x2, gbv[2]),
            "maskg": np.ascontiguousarray(maskg.reshape(17, P).T),
        })
        in_maps.append(m)
    return in_maps


def _run_cached_pjrt(nc, in_maps):
    """run_bass_via_pjrt equivalent with the traced/jitted executable cached
    across kernel() calls (run_bass_via_pjrt rebuilds it every call)."""
    import jax
    import numpy as _np
    from jax.sharding import Mesh, PartitionSpec
    from jax.experimental.shard_map import shard_map
    from concourse import bass2jax, mybir as _mb

    n_cores = len(in_maps)
    if "pjrt" not in _CACHE:
        bass2jax.install_neuronx_cc_hook()
        in_names, out_names, out_avals, zero_shapes = [], [], [], []
        for alloc in nc.m.functions[0].allocations:
            if not isinstance(alloc, _mb.MemoryLocationSet):
                continue
            name = alloc.memorylocations[0].name
            if alloc.kind == "ExternalInput":
                if nc.partition_id_tensor is None or \
                        name != nc.partition_id_tensor.name:
                    in_names.append(name)
            elif alloc.kind == "ExternalOutput":
                out_names.append(name)
                shape = tuple(alloc.tensor_shape)
                dtype = _mb.dt.np(alloc.dtype)
                out_avals.append(jax.core.ShapedArray(shape, dtype))
                zero_shapes.append((shape, dtype))
        n_params = len(in_names)
        all_names = in_names + out_names
        pid_name = nc.partition_id_tensor.name if nc.partition_id_tensor else None
        if pid_name is not None:
            all_names = all_names + [pid_name]

        def _body(*args):
            operands = list(args)
            if pid_name is not None:
                operands.append(bass2jax.partition_id_tensor())
            outs = bass2jax._bass_exec_p.bind(
                *operands,
                out_avals=tuple(out_avals),
                in_names=tuple(all_names),
                out_names=tuple(out_names),
                lowering_input_output_aliases=(),
                sim_require_finite=True,
                sim_require_nnan=True,
                nc=nc,
            )
            return tuple(outs)

        devices = jax.devices()[:n_cores]
        mesh = Mesh(_np.asarray(devices), ("core",))
        n_outs = len(out_names)
        sharded = jax.jit(
            shard_map(_body, mesh=mesh,
                      in_specs=(PartitionSpec("core"),) * (n_params + n_outs),
                      out_specs=(PartitionSpec("core"),) * n_outs,
                      check_rep=False),
            donate_argnums=tuple(range(n_params, n_params + n_outs)),
            keep_unused=True,
        )
        _CACHE["pjrt"] = (sharded, in_names, out_names, out_avals, zero_shapes)

    sharded, in_names, out_names, out_avals, zero_shapes = _CACHE["pjrt"]
    n_cores_ax = len(in_maps)
    concat_in = [
        _np.concatenate([_np.asarray(in_maps[c][nm]) for c in range(n_cores_ax)], axis=0)
        for nm in in_names
    ]
    concat_zeros = [
        _np.zeros((n_cores_ax * s[0], *s[1:]), d) for s, d in zero_shapes
    ]
    out_arrs = sharded(*concat_in, *concat_zeros)
    return [
        {nm: _np.asarray(out_arrs[i]).reshape(n_cores_ax, *out_avals[i].shape)[c]
         for i, nm in enumerate(out_names)}
        for c in range(n_cores_ax)
    ]


def kernel(**inputs):
    nc = _build()
    in_maps = _prep_host(inputs)
    try:
        results = _run_cached_pjrt(nc, in_maps)
    except Exception:
        _CACHE.pop("pjrt", None)
        res = run_bass_kernel_spmd(nc, in_maps, core_ids=list(range(8)))
        _CACHE["last_results"] = res
        results = res.results
    feat = np.zeros((4, C, 64, 64), np.float32)
    o1 = np.zeros((4, C, 64, 64), np.float32)
    o2 = np.zeros((4, C, 64, 64), np.float32)
    for core in range(8):
        s, half = core // 2, core % 2
        r = results[core]
        feat[s, :, 32 * half:32 * half + 32] = r["feat"]
        o1[s, :, 32 * half:32 * half + 32] = r["o1"]
        o2[s, :, 32 * half:32 * half + 32] = r["o2"]
    return (feat, o1, o2)



# revision 13
# speedup vs baseline: 1.6318x; 1.6318x over previous
"""Trainium2 Bass kernel for nn_CrossAtt (dual cross-attention + 3x3 conv + BN + ReLU).

Sharding: 8 cores = (sample s in 0..3) x (h-half in 0..1), no collectives.
Each core computes 32 output rows + 1-row halo (34 rows = 2176 queries).

v2 design (fp8 DoubleRow everywhere hot):
- Host ROTATES x per core so the query/residual window is always columns
  [0, 2176) of the rotated tensor (attention is permutation-invariant in the
  key dim; wrapped rows land in the masked fake-row slots). This kills the
  separate x?q/x?r input tensors.
- scores^T = x_fp8^T . qk where qk = Wk^T(q)  (Wk folded into the small q
  side: 256-deep fp8 DoubleRow contraction, 0.5 cyc/col).
- exp on ACT with scale=1/8 (qk stored x8 to stay in fp8-normal range) and
  bias=-2 (cancels in softmax; keeps exp in e4m3 range).
- AV: out^T[n, c] accumulated over 16 fp8 DoubleRow calls (256 m per call);
  softmax denominator S via parallel ones-column matmuls into col 256.
- epilogue: nt = (AV * recip(S)) * gamma*mask -> bf16, PE-transpose (bf16),
  cat = tp + xr (xr has gamma*bv folded on host).
- conv 3x3 in bf16, split into branch-1 half (overlapped with branch-2
  attention) + branch-2 half (tail), merged via SBUF f32 accumulator.
- outputs in bf16 (host upcasts); tolerance is 2e-2, errors here ~1e-3.
"""
import sys

if "/opt/trn_rl_repo" not in sys.path:
    sys.path.insert(0, "/opt/trn_rl_repo")

import numpy as np

import concourse.bass as bass
import concourse.bacc as bacc
import concourse.mybir as mybir
import concourse.tile as tile
from concourse.bass import ds, ts
from concourse.bass_utils import run_bass_kernel_spmd

F32 = mybir.dt.float32
BF16 = mybir.dt.bfloat16
F8 = mybir.dt.float8e4
DR = mybir.MatmulPerfMode.DoubleRow
EXP = mybir.ActivationFunctionType.Exp
RELU = mybir.ActivationFunctionType.Relu
EPS = 1e-5
P = 128
C = 256
M = 4096          # key/value positions
NQ = 2176         # query positions (34 rows * 64)
NCH = 17          # n-chunks of 128
NROWS = 35
WPAD = 66
CATW = NROWS * WPAD  # 2310
FEATW = 31 * 66 + 64  # 2110 featv span (out rows 1..32, cols 1..64)
SW = 16.0         # fp8 weight upscale
QKS = 8.0         # qk stored as 8x true
NB = [(i * 256, 256) for i in range(8)] + [(2048, 128)]
FWINS = [(0, 512), (512, 512), (1024, 512), (1536, 512), (2048, 62)]
CATBASE = 67      # featv pos 0 == cat pos 67 (row 1, col 1)

_CACHE = {}


def _declare_io(nc):
    t = {}
    inp = lambda name, shape, dt=F32: t.__setitem__(
        name, nc.dram_tensor(name, shape, dt, kind="ExternalInput"))
    out = lambda name, shape, dt=F32: t.__setitem__(
        name, nc.dram_tensor(name, shape, dt, kind="ExternalOutput"))
    for b in (1, 2):
        inp(f"xk{b}", [P, 2, M], F8)
        inp(f"xr{b}", [P, 2, NQ], BF16)
        inp(f"wq{b}", [P, 2, 32], F8)
        inp(f"wkq{b}", [32, 2, C], F8)
        inp(f"wv{b}", [P, 2, C], F8)
    inp("bq", [32, 2])
    inp("maskg", [P, NCH])
    inp("cinv", [P, 2])
    inp("cbeta", [P, 2])
    inp("wct", [P, 4, 9, 2, P], BF16)
    inp("identb", [P, P], BF16)
    out("o1", [P, 2, 32, WPAD], BF16)
    out("o2", [P, 2, 32, WPAD], BF16)
    out("feat", [P, 2, FEATW], BF16)
    return t


def _emit(nc, tc, t, ctx):
    sing = ctx.enter_context(tc.tile_pool(name="sing", bufs=1))
    xkp = ctx.enter_context(tc.tile_pool(name="xkp", bufs=2))
    xrp = ctx.enter_context(tc.tile_pool(name="xrp", bufs=2))
    qp = ctx.enter_context(tc.tile_pool(name="qp", bufs=1))
    qkp = ctx.enter_context(tc.tile_pool(name="qkp", bufs=2))
    vtp = ctx.enter_context(tc.tile_pool(name="vtp", bufs=2))
    exp_pool = ctx.enter_context(tc.tile_pool(name="exq", bufs=2))
    ntp = ctx.enter_context(tc.tile_pool(name="ntp", bufs=2))
    catp = ctx.enter_context(tc.tile_pool(name="catp", bufs=2))
    convp = ctx.enter_context(tc.tile_pool(name="convp", bufs=1))
    featp = ctx.enter_context(tc.tile_pool(name="featp", bufs=1))
    fprep = ctx.enter_context(tc.tile_pool(name="fprep", bufs=2))
    psc = ctx.enter_context(tc.tile_pool(name="psc", bufs=2, space="PSUM"))
    psav = ctx.enter_context(tc.tile_pool(name="psav", bufs=1, space="PSUM"))
    pstp = ctx.enter_context(tc.tile_pool(name="pstp", bufs=1, space="PSUM"))
    psconv = ctx.enter_context(tc.tile_pool(name="psconv", bufs=1, space="PSUM"))

    mm = nc.tensor.matmul

    # ---- constants / weights ----
    wq, wkq, wv, xk, xr = {}, {}, {}, {}, {}
    for b in (1, 2):
        wq[b] = sing.tile([P, 2, 32], F8, tag=f"wq{b}", name=f"wq{b}")
        nc.sync.dma_start(out=wq[b], in_=t[f"wq{b}"][:])
        wkq[b] = sing.tile([32, 2, C], F8, tag=f"wkq{b}", name=f"wkq{b}")
        nc.sync.dma_start(out=wkq[b], in_=t[f"wkq{b}"][:])
        wv[b] = sing.tile([P, 2, C], F8, tag=f"wv{b}", name=f"wv{b}")
        nc.sync.dma_start(out=wv[b], in_=t[f"wv{b}"][:])
    bq_sb = sing.tile([32, 2], F32, tag="bq")
    nc.sync.dma_start(out=bq_sb, in_=t["bq"][:])
    maskg_sb = sing.tile([P, NCH], F32, tag="maskg")
    nc.sync.dma_start(out=maskg_sb, in_=t["maskg"][:])
    cinv_sb = sing.tile([P, 2], F32, tag="cinv")
    nc.sync.dma_start(out=cinv_sb, in_=t["cinv"][:])
    cbeta_sb = sing.tile([P, 2], F32, tag="cbeta")
    nc.sync.dma_start(out=cbeta_sb, in_=t["cbeta"][:])
    identb = sing.tile([P, P], BF16, tag="identb")
    nc.sync.dma_start(out=identb, in_=t["identb"][:])
    wct = sing.tile([P, 4, 9, 2, P], BF16, tag="wct")
    nc.sync.dma_start(out=wct, in_=t["wct"][:])
    for b in (1, 2):
        xk[b] = xkp.tile([P, 2, M], F8, tag="xk", name=f"xk{b}")
        for h in range(2):
            nc.sync.dma_start(out=xk[b][:, h], in_=t[f"xk{b}"][:, h])
    for b in (1, 2):
        xr[b] = xrp.tile([P, 2, NQ], BF16, tag="xr", name=f"xr{b}")
        nc.sync.dma_start(out=xr[b], in_=t[f"xr{b}"][:])

    ones_pair = sing.tile([P, 2, 1], F8, tag="ones")
    nc.vector.memset(ones_pair, 1.0)
    ebias = sing.tile([P, 1], F32, tag="ebias")
    nc.vector.memset(ebias, -2.0)

    cat = {1: catp.tile([P, 2, CATW], BF16, tag="cat", name="cat1"),
           2: catp.tile([P, 2, CATW], BF16, tag="cat", name="cat2")}
    nc.vector.memset(cat[1], 0.0)
    nc.gpsimd.memset(cat[2], 0.0)
    convsb = convp.tile([P, 2, FEATW], F32, tag="convsb")
    featv = featp.tile([P, 2, FEATW], BF16, tag="featv")

    # ---- q projection: q_sb[p, b, n] fp8 (q channels kt*32+p, kt=branch) ----
    q_sb = qp.tile([32, 2, NQ], F8, tag="q")
    for b in (1, 2):
        for w0 in (0, 1024):
            ps = psc.tile([P, 1024], F32, tag="sc", name="qps")
            for u in range(4):
                mm(ps[0:32, ds(u * 256, 256)], wq[b][:],
                   xk[b][:, :, ds(w0 + u * 256, 256)],
                   start=True, stop=True, perf_mode=DR)
            nc.vector.tensor_scalar(
                out=q_sb[:, b - 1, ds(w0, 1024)], in0=ps[0:32, :],
                scalar1=1.0 / SW, scalar2=bq_sb[:, ds(b - 1, 1)],
                op0=mybir.AluOpType.mult, op1=mybir.AluOpType.add)
        ps = psc.tile([P, 1024], F32, tag="sc", name="qps2")
        mm(ps[0:32, 0:128], wq[b][:], xk[b][:, :, ds(2048, 128)],
           start=True, stop=True, perf_mode=DR)
        nc.vector.tensor_scalar(
            out=q_sb[:, b - 1, ds(2048, 128)], in0=ps[0:32, 0:128],
            scalar1=1.0 / SW, scalar2=bq_sb[:, ds(b - 1, 1)],
            op0=mybir.AluOpType.mult, op1=mybir.AluOpType.add)

    # ---- qk = Wk^T q, stored x8: qk[p, cch, n] fp8 ----
    qk = {}
    for b in (1, 2):
        qk[b] = qkp.tile([P, 2, NQ], F8, tag="qk", name=f"qk{b}")
        eng = [nc.vector, nc.vector]  # gpsimd cannot read PSUM
        for cch in range(2):
            for gi, w0 in enumerate((0, 1024)):
                ps = psc.tile([P, 1024], F32, tag="sc", name="qkps")
                for u in range(4):
                    mm(ps[:, ds(u * 256, 256)], wkq[b][:, :, ts(cch, P)],
                       q_sb[:, :, ds(w0 + u * 256, 256)],
                       start=True, stop=True, perf_mode=DR)
                eng[cch].tensor_scalar_mul(qk[b][:, cch, ds(w0, 1024)],
                                           ps[:], QKS / SW)
            ps = psc.tile([P, 1024], F32, tag="sc", name="qkps2")
            mm(ps[:, 0:128], wkq[b][:, :, ts(cch, P)],
               q_sb[:, :, ds(2048, 128)],
               start=True, stop=True, perf_mode=DR)
            eng[cch].tensor_scalar_mul(qk[b][:, cch, ds(2048, 128)],
                                       ps[:, 0:128], QKS / SW)

    # ---- vt[p, mi, c] fp8: vt = x^T wv (bias bv folded into xr on host) ----
    vt = {}
    for b in (1, 2):
        vt[b] = vtp.tile([P, 32, C], F8, tag="vt", name=f"vt{b}")
        for tg in range(8):
            ps = psc.tile([P, 1024], F32, tag="sc", name="vtps")
            for u in range(4):
                mi = 4 * tg + u
                mm(ps[:, ds(u * 256, 256)], xk[b][:, :, ts(mi, P)],
                   wv[b][:], start=True, stop=True, perf_mode=DR)
            nc.vector.tensor_scalar_mul(
                vt[b][:, ds(4 * tg, 4), :],
                ps[:].rearrange("p (u c) -> p u c", u=4), 1.0 / SW)

    # ---- conv piece queue (emitted interleaved with attention) ----
    conv_state = {"q": []}

    def conv_half_quantum(half, win_i, oc, icc_rel):
        """Emit 9 tap matmuls for one (window, oc, icc) of a conv half."""
        ws, ww = FWINS[win_i]
        icc = 2 * (half - 1) + icc_rel
        cb = cat[half]
        pst = conv_state["ps"]
        for tap in range(9):
            off = (tap // 3 - 1) * WPAD + (tap % 3 - 1)
            rhs = cb[:, icc_rel, ds(CATBASE + ws + off, ww)]
            mm(pst[:, 0:ww], wct[:, icc, tap, oc, :], rhs,
               start=(icc_rel == 0 and tap == 0),
               stop=(icc_rel == 1 and tap == 8))

    def conv_half1_pieces():
        for win_i in range(5):
            for oc in range(2):
                def alloc(win_i=win_i, oc=oc):
                    conv_state["ps"] = psconv.tile([P, 512], F32, tag="cps",
                                                   name=f"cv{win_i}_{oc}")
                for icc_rel in range(2):
                    if icc_rel == 0:
                        yield alloc
                    yield (lambda w=win_i, o=oc, i=icc_rel:
                           conv_half_quantum(1, w, o, i))

                def flush(win_i=win_i, oc=oc):
                    ws, ww = FWINS[win_i]
                    nc.vector.tensor_copy(
                        out=convsb[:, oc, ds(ws, ww)],
                        in_=conv_state["ps"][:, 0:ww])
                yield flush

    def emit_conv_piece():
        if conv_state["q"]:
            conv_state["q"].pop(0)()

    # ---- attention ----
    def av_slice(b, ex, n0, nw, g):
        """Emit 1/8 of the AV matmuls for a finished exp block."""
        nsub = nw // P
        if nsub == 2:
            sub, k0 = (0, 4 * g) if g < 4 else (1, 4 * (g - 4))
            kcount = 4
        else:
            sub, k0, kcount = 0, 2 * g, 2
        if k0 == 0:
            conv_state[f"av{b}"] = psav.tile([P, 1024], F32, tag="av",
                                             name=f"av{b}_{n0}_{sub}")
        av = conv_state[f"av{b}"]
        for kk in range(k0, k0 + kcount):
            lhsT = ex[:, ds(2 * kk, 2), ds(sub * P, P)]
            # AV accumulates in bank 0 (cols 0:256); the softmax denominator
            # S accumulates in bank 1 (col 512) so the two start=True groups
            # zero-mark disjoint 2KB PSUM regions.
            mm(av[:, 0:256], lhsT, vt[b][:, ds(2 * kk, 2), :],
               start=(kk == 0), stop=(kk == 15), perf_mode=DR,
               skip_group_check=True)
            mm(av[:, ds(512, 1)], lhsT, ones_pair[:],
               start=(kk == 0), stop=(kk == 15), perf_mode=DR,
               skip_group_check=True)
        return av if kk == 15 else None

    def epilogue(b, n0, sub, av):
        nch = n0 // P + sub
        rs = ntp.tile([P, 1], F32, tag="rs")
        nc.vector.reciprocal(rs, av[:, ds(512, 1)])
        nt = ntp.tile([P, 256], BF16, tag="nt")
        nc.vector.tensor_scalar(
            out=nt, in0=av[:, 0:256], scalar1=rs,
            scalar2=maskg_sb[:, ds(nch, 1)],
            op0=mybir.AluOpType.mult, op1=mybir.AluOpType.mult)
        # full-bank tile (2KB) so every psum tile stays bank-aligned
        tpfull = pstp.tile([P, 1024], BF16, tag="tp", name="tpfull")
        tp = tpfull[:, 0:256]
        for cc in range(2):
            nc.tensor.transpose(tp[:, ts(cc, P)], nt[:, ts(cc, P)], identb)
        catv = cat[b][:].rearrange("p cc (r w) -> p cc r w", w=WPAD)
        nc.vector.tensor_tensor(
            out=catv[:, :, ds(2 * nch, 2), ds(1, 64)],
            in0=tp.rearrange("p (cc r w) -> p cc r w", cc=2, w=64),
            in1=xr[b][:, :, ds(nch * P, P)].rearrange(
                "p cc (r w) -> p cc r w", w=64),
            op=mybir.AluOpType.add)

    for b in (1, 2):
        if b == 2:
            conv_state["q"] = list(conv_half1_pieces())
        prev = None
        for j, (n0, nw) in enumerate(NB):
            ex = exp_pool.tile([P, 32, 256], F8, tag="ex", name=f"ex{b}_{j}")
            for g in range(8):
                ps = psc.tile([P, 1024], F32, tag="sc", name=f"sc{b}_{j}_{g}")
                for u in range(4):
                    mi = 4 * g + u
                    mm(ps[:, ds(u * nw, nw)], xk[b][:, :, ts(mi, P)],
                       qk[b][:, :, ds(n0, nw)],
                       start=True, stop=True, perf_mode=DR)
                nc.scalar.activation(
                    ex[:, ds(4 * g, 4), 0:nw], ps[:, 0:4 * nw],
                    EXP, bias=ebias, scale=0.125)
                if prev is not None:
                    pex, pn0, pnw = prev
                    av = av_slice(b, pex, pn0, pnw, g)
                    if av is not None:
                        sub = 0 if (pnw == P or g < 4) else 1
                        epilogue(b, pn0, sub, av)
                if b == 2:
                    emit_conv_piece()
            prev = (ex, n0, nw)
        # drain the final block's AV + epilogue
        pex, pn0, pnw = prev
        for g in range(8):
            av = av_slice(b, pex, pn0, pnw, g)
            if av is not None:
                sub = 0 if (pnw == P or g < 4) else 1
                epilogue(b, pn0, sub, av)
            if b == 2:
                emit_conv_piece()
        # branch output
        ov = cat[b][:].rearrange("p cc (r w) -> p cc r w", w=WPAD)
        nc.sync.dma_start(out=t[f"o{b}"][:], in_=ov[:, :, ds(1, 32), :])
    while conv_state["q"]:
        emit_conv_piece()

    # ---- conv half-2 (cat2) + merge + BN/ReLU ----
    for win_i in range(5):
        ws, ww = FWINS[win_i]
        for oc in range(2):
            pool = psc if win_i % 2 == 0 else psconv
            pst = pool.tile([P, 512] if pool is psconv else [P, 1024],
                            F32, tag="cps" if pool is psconv else "sc",
                            name=f"c2_{win_i}_{oc}")
            conv_state["ps"] = pst
            for icc_rel in range(2):
                conv_half_quantum(2, win_i, oc, icc_rel)
            fp = fprep.tile([P, 512], BF16, tag="fp")
            nc.vector.tensor_tensor(out=fp[:, 0:ww], in0=pst[:, 0:ww],
                                    in1=convsb[:, oc, ds(ws, ww)],
                                    op=mybir.AluOpType.add)
            nc.scalar.activation(featv[:, oc, ds(ws, ww)], fp[:, 0:ww],
                                 RELU, bias=cbeta_sb[:, ds(oc, 1)],
                                 scale=cinv_sb[:, ds(oc, 1)])
    nc.sync.dma_start(out=t["feat"][:], in_=featv[:])


def _build():
    if "nc" in _CACHE:
        return _CACHE["nc"]
    nc = bacc.Bacc(None, target_bir_lowering=False)
    t = _declare_io(nc)
    from contextlib import ExitStack
    with tile.TileContext(nc) as tc, ExitStack() as ctx:
        _emit(nc, tc, t, ctx)
    nc.finalize()
    _CACHE["nc"] = nc
    return nc


def _prep_host(inputs):
    import ml_dtypes
    F8NP = ml_dtypes.float8_e4m3
    BFNP = ml_dtypes.bfloat16
    d = {k: np.ascontiguousarray(np.asarray(v, np.float32))
         for k, v in inputs.items()}
    gamma = float(d["gamma"].reshape(-1)[0])
    inv = d["bn_scale"] / np.sqrt(d["bn_var"] + EPS)
    beta = d["bn_bias"] - d["bn_mean"] * inv

    def f8(x):
        return np.clip(x, -200.0, 200.0).astype(F8NP)

    shared = {
        "bq": np.stack([d["bq1"], d["bq2"]], axis=1),
        "cinv": np.ascontiguousarray(inv.reshape(2, P).T),
        "cbeta": np.ascontiguousarray(beta.reshape(2, P).T),
        "identb": np.eye(P, dtype=BFNP),
    }
    for b in (1, 2):
        wqb = d[f"wq{b}"]      # [32, 256]
        wkb = d[f"wk{b}"]      # [64, 256]
        wvb = d[f"wv{b}"]      # [256, 256]
        # wq: [p, kc, o] = 16*wq[o, kc*128+p]
        shared[f"wq{b}"] = f8(
            (SW * wqb.T).reshape(2, P, 32).transpose(1, 0, 2))
        # wkq: [p, kt, c] = 16*wk[kt*32+p, c]
        shared[f"wkq{b}"] = f8(SW * wkb.reshape(2, 32, C).transpose(1, 0, 2))
        # wv: [p, kc, co] = 16*wv[co, kc*128+p]
        shared[f"wv{b}"] = f8(
            (SW * wvb.T).reshape(2, P, C).transpose(1, 0, 2))
    # wct: [ic_p, icc, tap, occ, oc] = w_cat[occ*128+oc, icc*128+ic_p, ky, kx]
    wcat = d["w_cat"]  # [256, 512, 3, 3]
    shared["wct"] = np.ascontiguousarray(
        wcat.reshape(2, P, 4, P, 9).transpose(3, 2, 4, 0, 1)).astype(BFNP)

    gbv = {1: gamma * d["bv1"], 2: gamma * d["bv2"]}
    in_maps = []
    for core in range(8):
        s, half = core // 2, core % 2
        h0 = 32 * half
        shift = (h0 - 1) * 64
        m = dict(shared)
        fake_row = 0 if half == 0 else 33
        maskg = np.full((34, 64), gamma, np.float32)
        maskg[fake_row] = 0.0
        m["maskg"] = np.ascontiguousarray(
            maskg.reshape(NCH, P).T)
        for b in (1, 2):
            x = d[f"input{b}"][s].reshape(C, M)
            x_rot = np.roll(x, -shift, axis=1)
            m[f"xk{b}"] = f8(x_rot.reshape(2, P, M).transpose(1, 0, 2))
            xw = x_rot[:, :NQ] + gbv[b][:, None]
            xw = xw.reshape(C, 34, 64).copy()
            xw[:, fake_row, :] = 0.0
            m[f"xr{b}"] = np.ascontiguousarray(
                xw.reshape(2, P, NQ).transpose(1, 0, 2)).astype(BFNP)
        in_maps.append(m)
    return in_maps


def _run_cached_pjrt(nc, in_maps):
    import jax
    import numpy as _np
    from jax.sharding import Mesh, PartitionSpec
    from jax.experimental.shard_map import shard_map
    from concourse import bass2jax, mybir as _mb

    n_cores = len(in_maps)
    if "pjrt" not in _CACHE:
        bass2jax.install_neuronx_cc_hook()
        in_names, out_names, out_avals, zero_shapes = [], [], [], []
        for alloc in nc.m.functions[0].allocations:
            if not isinstance(alloc, _mb.MemoryLocationSet):
                continue
            name = alloc.memorylocations[0].name
            if alloc.kind == "ExternalInput":
                if nc.partition_id_tensor is None or \
                        name != nc.partition_id_tensor.name:
                    in_names.append(name)
            elif alloc.kind == "ExternalOutput":
                out_names.append(name)
                shape = tuple(alloc.tensor_shape)
                dtype = _mb.dt.np(alloc.dtype)
                out_avals.append(jax.core.ShapedArray(shape, dtype))
                zero_shapes.append((shape, dtype))
        n_params = len(in_names)
        all_names = in_names + out_names
        pid_name = nc.partition_id_tensor.name if nc.partition_id_tensor else None
        if pid_name is not None:
            all_names = all_names + [pid_name]

        def _body(*args):
            operands = list(args)
            if pid_name is not None:
                operands.append(bass2jax.partition_id_tensor())
            outs = bass2jax._bass_exec_p.bind(
                *operands,
                out_avals=tuple(out_avals),
                in_names=tuple(all_names),
                out_names=tuple(out_names),
                lowering_input_output_aliases=(),
                sim_require_finite=True,
                sim_require_nnan=True,
                nc=nc,
            )
            return tuple(outs)

        devices = jax.devices()[:n_cores]
        mesh = Mesh(_np.asarray(devices), ("core",))
        n_outs = len(out_names)
        sharded = jax.jit(
            shard_map(_body, mesh=mesh,
                      in_specs=(PartitionSpec("core"),) * (n_params + n_outs),
                      out_specs=(PartitionSpec("core"),) * n_outs,
                      check_rep=False),
            donate_argnums=tuple(range(n_params, n_params + n_outs)),
            keep_unused=True,
        )
        _CACHE["pjrt"] = (sharded, in_names, out_names, out_avals, zero_shapes)

    sharded, in_names, out_names, out_avals, zero_shapes = _CACHE["pjrt"]
    n_cores_ax = len(in_maps)
    concat_in = [
        _np.concatenate([_np.asarray(in_maps[c][nm]) for c in range(n_cores_ax)], axis=0)
        for nm in in_names
    ]
    concat_zeros = [
        _np.zeros((n_cores_ax * s[0], *s[1:]), d) for s, d in zero_shapes
    ]
    out_arrs = sharded(*concat_in, *concat_zeros)
    return [
        {nm: _np.asarray(out_arrs[i]).reshape(n_cores_ax, *out_avals[i].shape)[c]
         for i, nm in enumerate(out_names)}
        for c in range(n_cores_ax)
    ]


def kernel(**inputs):
    nc = _build()
    in_maps = _prep_host(inputs)
    try:
        results = _run_cached_pjrt(nc, in_maps)
    except Exception:
        _CACHE.pop("pjrt", None)
        res = run_bass_kernel_spmd(nc, in_maps, core_ids=list(range(8)))
        _CACHE["last_results"] = res
        results = res.results
    feat = np.zeros((4, C, 64, 64), np.float32)
    o1 = np.zeros((4, C, 64, 64), np.float32)
    o2 = np.zeros((4, C, 64, 64), np.float32)
    for core in range(8):
        s, half = core // 2, core % 2
        r = results[core]
        sl = slice(32 * half, 32 * half + 32)
        # o outputs: [P, 2, 32, 66] bf16, cols 1..64 are data
        for name, dst in (("o1", o1), ("o2", o2)):
            v = np.asarray(r[name]).astype(np.float32)[:, :, :, 1:65]
            dst[s, :, sl] = v.transpose(1, 0, 2, 3).reshape(C, 32, 64)
        fv = np.asarray(r["feat"]).astype(np.float32)  # [P, 2, 2110]
        fimg = np.zeros((P, 2, 32, 64), np.float32)
        flat = fv.reshape(P, 2, FEATW)
        for rr in range(32):
            fimg[:, :, rr, :] = flat[:, :, rr * 66:rr * 66 + 64]
        feat[s, :, sl] = fimg.transpose(1, 0, 2, 3).reshape(C, 32, 64)
    return (feat, o1, o2)


# revision 36
# speedup vs baseline: 1.9210x; 1.1772x over previous
"""Trainium2 Bass kernel for nn_CrossAtt (dual cross-attention + 3x3 conv + BN + ReLU).

Sharding: 8 cores = (sample s in 0..3) x (h-half in 0..1), no collectives.
Each core computes 32 output rows + 1-row halo (34 rows = 2176 queries).

v2 design (fp8 DoubleRow everywhere hot):
- Host ROTATES x per core so the query/residual window is always columns
  [0, 2176) of the rotated tensor (attention is permutation-invariant in the
  key dim; wrapped rows land in the masked fake-row slots). This kills the
  separate x?q/x?r input tensors.
- scores^T = x_fp8^T . qk where qk = Wk^T(q)  (Wk folded into the small q
  side: 256-deep fp8 DoubleRow contraction, 0.5 cyc/col).
- exp on ACT with scale=1/8 (qk stored x8 to stay in fp8-normal range) and
  bias=-2 (cancels in softmax; keeps exp in e4m3 range).
- AV: out^T[n, c] accumulated over 16 fp8 DoubleRow calls (256 m per call);
  softmax denominator S via parallel ones-column matmuls into col 256.
- epilogue: nt = (AV * recip(S)) * gamma*mask -> bf16, PE-transpose (bf16),
  cat = tp + xr (xr has gamma*bv folded on host).
- conv 3x3 in bf16, split into branch-1 half (overlapped with branch-2
  attention) + branch-2 half (tail), merged via SBUF f32 accumulator.
- outputs in bf16 (host upcasts); tolerance is 2e-2, errors here ~1e-3.
"""
import sys

if "/opt/trn_rl_repo" not in sys.path:
    sys.path.insert(0, "/opt/trn_rl_repo")

import numpy as np

import concourse.bass as bass
import concourse.bacc as bacc
import concourse.mybir as mybir
import concourse.tile as tile
from concourse.bass import ds, ts
from concourse.bass_utils import run_bass_kernel_spmd

F32 = mybir.dt.float32
BF16 = mybir.dt.bfloat16
F8 = mybir.dt.float8e4
DR = mybir.MatmulPerfMode.DoubleRow
EXP = mybir.ActivationFunctionType.Exp
RELU = mybir.ActivationFunctionType.Relu
EPS = 1e-5
P = 128
C = 256
M = 4096          # key/value positions
NQ = 2176         # query positions (34 rows * 64)
NCH = 17          # n-chunks of 128
NROWS = 35
WPAD = 66
CATW = NROWS * WPAD  # 2310
FEATW = 31 * 66 + 64  # 2110 featv span (out rows 1..32, cols 1..64)
SW = 16.0         # fp8 weight upscale
QKS = 8.0         # qk stored as 8x true
NB = [(i * 256, 256) for i in range(8)] + [(2048, 128)]
FWINS = [(0, 512), (512, 512), (1024, 512), (1536, 512), (2048, 62)]
CATBASE = 67      # featv pos 0 == cat pos 67 (row 1, col 1)

_CACHE = {}


def _declare_io(nc):
    t = {}
    inp = lambda name, shape, dt=F32: t.__setitem__(
        name, nc.dram_tensor(name, shape, dt, kind="ExternalInput"))
    out = lambda name, shape, dt=F32: t.__setitem__(
        name, nc.dram_tensor(name, shape, dt, kind="ExternalOutput"))
    for b in (1, 2):
        inp(f"xk{b}", [P, 2, M], F8)
        inp(f"xr{b}", [P, 2, NQ], BF16)
        # wm{b}{i}: fused (Wk_b^T Wq_i) matrices, [cin_p, cin_kc, cout], x256
        inp(f"wm{b}1", [P, 2, C], F8)
        inp(f"wm{b}2", [P, 2, C], F8)
        inp(f"wv{b}", [P, 2, C], F8)
    inp("vq", [P, 2, 2])  # Wk_b^T bq per (c_p, cch, branch)
    inp("maskg", [P, NCH])
    inp("cinv", [P, 2])
    inp("cbeta", [P, 2])
    inp("wct", [P, 4, 9, 2, P], BF16)
    inp("identb", [P, P], BF16)
    out("o1", [P, 2, 32, WPAD], BF16)
    out("o2", [P, 2, 32, WPAD], BF16)
    out("feat", [P, 2, FEATW], BF16)
    return t


def _emit(nc, tc, t, ctx):
    sing = ctx.enter_context(tc.tile_pool(name="sing", bufs=1))
    xkp = ctx.enter_context(tc.tile_pool(name="xkp", bufs=2))
    xrp = ctx.enter_context(tc.tile_pool(name="xrp", bufs=2))
    qkp = ctx.enter_context(tc.tile_pool(name="qkp", bufs=2))
    vtp = ctx.enter_context(tc.tile_pool(name="vtp", bufs=2))
    exp_pool = ctx.enter_context(tc.tile_pool(name="exq", bufs=3))
    ntp = ctx.enter_context(tc.tile_pool(name="ntp", bufs=4))
    catp = ctx.enter_context(tc.tile_pool(name="catp", bufs=2))
    convp = ctx.enter_context(tc.tile_pool(name="convp", bufs=1))
    featp = ctx.enter_context(tc.tile_pool(name="featp", bufs=1))
    fprep = ctx.enter_context(tc.tile_pool(name="fprep", bufs=2))
    psc = ctx.enter_context(tc.tile_pool(name="psc", bufs=2, space="PSUM"))
    psav = ctx.enter_context(tc.tile_pool(name="psav", bufs=1, space="PSUM"))
    psS = ctx.enter_context(tc.tile_pool(name="psS", bufs=1, space="PSUM"))
    pstp = ctx.enter_context(tc.tile_pool(name="pstp", bufs=1, space="PSUM"))
    psconv = ctx.enter_context(tc.tile_pool(name="psconv", bufs=1, space="PSUM"))

    mm = nc.tensor.matmul

    # ---- inputs first (xk gates the qk chain), big conv weights last ----
    wm, wv, xk, xr = {}, {}, {}, {}
    for b in (1, 2):
        xk[b] = xkp.tile([P, 2, M], F8, tag="xk", name=f"xk{b}")
        for h in range(2):
            nc.sync.dma_start(out=xk[b][:, h], in_=t[f"xk{b}"][:, h])
    for b in (1, 2):
        for i in (1, 2):
            wm[(b, i)] = sing.tile([P, 2, C], F8, tag=f"wm{b}{i}",
                                   name=f"wm{b}{i}")
            nc.sync.dma_start(out=wm[(b, i)], in_=t[f"wm{b}{i}"][:])
        wv[b] = sing.tile([P, 2, C], F8, tag=f"wv{b}", name=f"wv{b}")
        nc.sync.dma_start(out=wv[b], in_=t[f"wv{b}"][:])
    vq_sb = sing.tile([P, 2, 2], F32, tag="vq")
    nc.sync.dma_start(out=vq_sb, in_=t["vq"][:])
    maskg_sb = sing.tile([P, NCH], F32, tag="maskg")
    nc.sync.dma_start(out=maskg_sb, in_=t["maskg"][:])
    cinv_sb = sing.tile([P, 2], F32, tag="cinv")
    nc.sync.dma_start(out=cinv_sb, in_=t["cinv"][:])
    cbeta_sb = sing.tile([P, 2], F32, tag="cbeta")
    nc.sync.dma_start(out=cbeta_sb, in_=t["cbeta"][:])
    identb = sing.tile([P, P], BF16, tag="identb")
    nc.sync.dma_start(out=identb, in_=t["identb"][:])
    for b in (1, 2):
        xr[b] = xrp.tile([P, 2, NQ], BF16, tag="xr", name=f"xr{b}")
        nc.sync.dma_start(out=xr[b], in_=t[f"xr{b}"][:])
    wct = sing.tile([P, 4, 9, 2, P], BF16, tag="wct")
    nc.sync.dma_start(out=wct, in_=t["wct"][:])

    ones_pair = sing.tile([P, 2, 1], F8, tag="ones")
    nc.vector.memset(ones_pair, 1.0)
    ebias = sing.tile([P, 1], F32, tag="ebias")
    nc.vector.memset(ebias, -2.0)

    cat = {1: catp.tile([P, 2, CATW], BF16, tag="cat", name="cat1"),
           2: catp.tile([P, 2, CATW], BF16, tag="cat", name="cat2")}
    nc.vector.memset(cat[1], 0.0)
    nc.gpsimd.memset(cat[2], 0.0)
    convsb = convp.tile([P, 2, FEATW], F32, tag="convsb")
    featv = featp.tile([P, 2, FEATW], BF16, tag="featv")

    # ---- fused qk_b = (Wk_b^T Wq1) x1 + (Wk_b^T Wq2) x2 + Wk_b^T bq ----
    # wm stored x256, so psum = 256*qk_true; cast by 1/32 -> qk_sb = 8*qk.
    qk = {1: qkp.tile([P, 2, NQ], F8, tag="qk", name="qk1"),
          2: qkp.tile([P, 2, NQ], F8, tag="qk", name="qk2")}
    vt = {1: vtp.tile([P, 32, C], F8, tag="vt", name="vt1"),
          2: vtp.tile([P, 32, C], F8, tag="vt", name="vt2")}

    def qk_group(b, cch, w0, ww, deferred=False):
        # deferred groups use the (idle) conv psum bank in 512-wide halves
        # so their late DVE cast never blocks the scores/exp psc rotation
        if deferred:
            # one 512-wide half-group (psum + cast) per call, so the single
            # conv-bank WAR resolves a full slot before the next half
            hw_ = min(512, ww)
            ps = psconv.tile([P, 512], F32, tag="cps",
                             name=f"qkd{b}{cch}{w0}")
            for u in range(max(1, hw_ // 256)):
                win = ds(w0 + u * 256, min(256, hw_))
                dst = ps[:, ds(u * 256, min(256, hw_))]
                for i in (1, 2):
                    mm(dst, wm[(b, i)][:, :, ts(cch, P)],
                       xk[i][:, :, win],
                       start=(i == 1), stop=(i == 2), perf_mode=DR)
            nc.vector.tensor_scalar(
                out=qk[b][:, cch, ds(w0, hw_)], in0=ps[:, 0:hw_],
                scalar1=1.0 / 32.0, scalar2=vq_sb[:, cch, ds(b - 1, 1)],
                op0=mybir.AluOpType.mult, op1=mybir.AluOpType.add)
            return
        ps = psc.tile([P, 1024], F32, tag="sc", name=f"qkp{b}{cch}{w0}")
        for u in range(max(1, ww // 256)):
            win = ds(w0 + u * 256, min(256, ww))
            dst = ps[:, ds(u * 256, min(256, ww))]
            for i in (1, 2):
                mm(dst, wm[(b, i)][:, :, ts(cch, P)], xk[i][:, :, win],
                   start=(i == 1), stop=(i == 2), perf_mode=DR)
        nc.vector.tensor_scalar(
            out=qk[b][:, cch, ds(w0, ww)], in0=ps[:, 0:ww],
            scalar1=1.0 / 32.0, scalar2=vq_sb[:, cch, ds(b - 1, 1)],
            op0=mybir.AluOpType.mult, op1=mybir.AluOpType.add)

    def vt_half(b, tg, h):
        # deferred-only: runs inside branch-1's attention slots on psconv
        ps = psconv.tile([P, 512], F32, tag="cps", name=f"vtd{b}{tg}{h}")
        for u in range(2):
            mi = 4 * tg + 2 * h + u
            mm(ps[:, ds(u * 256, 256)], xk[b][:, :, ts(mi, P)],
               wv[b][:], start=True, stop=True, perf_mode=DR)
        nc.vector.tensor_scalar_mul(
            vt[b][:, ds(4 * tg + 2 * h, 2), :],
            ps[:].rearrange("p (u c) -> p u c", u=2), 1.0 / SW)

    def vt_pieces(b):
        for tg in range(8):
            for h in range(2):
                yield (lambda b=b, tg=tg, h=h: vt_half(b, tg, h))

    def proj_pieces(b, deferred=False):
        for cch in range(2):
            if deferred:
                for w0 in (0, 512, 1024, 1536):
                    yield (lambda b=b, c=cch, w=w0:
                           qk_group(b, c, w, 512, True))
                yield (lambda b=b, c=cch: qk_group(b, c, 2048, 128, True))
            else:
                for w0 in (0, 1024):
                    yield (lambda b=b, c=cch, w=w0: qk_group(b, c, w, 1024))
                yield (lambda b=b, c=cch: qk_group(b, c, 2048, 128))

    # branch-1 qk upfront (gates the first exp); everything else deferred
    # into branch-1's attention g-slots.
    for piece in proj_pieces(1):
        piece()

    # ---- conv piece queue (emitted interleaved with attention) ----
    conv_state = {"q": []}

    def conv_half_quantum(half, win_i, oc, icc_rel):
        """Emit 9 tap matmuls for one (window, oc, icc) of a conv half."""
        ws, ww = FWINS[win_i]
        icc = 2 * (half - 1) + icc_rel
        cb = cat[half]
        pst = conv_state["ps"]
        for tap in range(9):
            off = (tap // 3 - 1) * WPAD + (tap % 3 - 1)
            rhs = cb[:, icc_rel, ds(CATBASE + ws + off, ww)]
            mm(pst[:, 0:ww], wct[:, icc, tap, oc, :], rhs,
               start=(icc_rel == 0 and tap == 0),
               stop=(icc_rel == 1 and tap == 8))

    def conv_half1_pieces():
        for win_i in range(5):
            for oc in range(2):
                def alloc(win_i=win_i, oc=oc):
                    conv_state["ps"] = psconv.tile([P, 512], F32, tag="cps",
                                                   name=f"cv{win_i}_{oc}")
                for icc_rel in range(2):
                    if icc_rel == 0:
                        yield alloc
                    yield (lambda w=win_i, o=oc, i=icc_rel:
                           conv_half_quantum(1, w, o, i))

                def flush(win_i=win_i, oc=oc):
                    ws, ww = FWINS[win_i]
                    nc.vector.tensor_copy(
                        out=convsb[:, oc, ds(ws, ww)],
                        in_=conv_state["ps"][:, 0:ww])
                yield flush

    def emit_conv_piece():
        if conv_state["q"]:
            conv_state["q"].pop(0)()

    # ---- attention ----
    # One persistent bank each: AV ping-pongs between its two halves per
    # chunk, S rotates over 4 columns. start=True zero-marks a whole 2KB
    # bank, but marked bytes keep their data until the next matmul WRITE —
    # and every region's next write is its own start=True group, so mixed
    # tenancy is safe while it doubles the effective AV buffering.
    avt = psav.tile([P, 512], F32, tag="avt", name="avt")
    stile = psS.tile([P, 512], F32, tag="stile", name="stile")
    conv_state["chunk"] = 0

    def av_slice(b, ex, n0, nw, g):
        """Emit 1/8 of the AV matmuls for a finished exp block."""
        nsub = nw // P
        if nsub == 2:
            sub, k0 = (0, 4 * g) if g < 4 else (1, 4 * (g - 4))
            kcount = 4
        else:
            sub, k0, kcount = 0, 2 * g, 2
        k = conv_state["chunk"]
        av = avt[:, ds(256 * (k % 2), 256)]
        sv = stile[:, ds(k % 4, 1)]
        for kk in range(k0, k0 + kcount):
            lhsT = ex[:, ds(2 * kk, 2), ds(sub * P, P)]
            mm(av, lhsT, vt[b][:, ds(2 * kk, 2), :],
               start=(kk == 0), stop=(kk == 15), perf_mode=DR,
               skip_group_check=True)
            mm(sv, lhsT, ones_pair[:],
               start=(kk == 0), stop=(kk == 15), perf_mode=DR,
               skip_group_check=True)
        if kk == 15:
            conv_state["chunk"] = k + 1
            return (av, sv)
        return None

    def epilogue(b, n0, sub, avsv):
        """Stage 1: free the AV psum half fast. Returns stage 2 closure."""
        av, sv = avsv
        nch = n0 // P + sub
        avs = ntp.tile([P, 256], F32, tag="avs")
        nc.vector.tensor_copy(out=avs, in_=av)
        rs = ntp.tile([P, 1], F32, tag="rs")
        nc.vector.reciprocal(rs, sv)

        def stage2(b=b, nch=nch, avs=avs, rs=rs):
            nt = ntp.tile([P, 256], BF16, tag="nt")
            nc.vector.tensor_scalar(
                out=nt, in0=avs, scalar1=rs,
                scalar2=maskg_sb[:, ds(nch, 1)],
                op0=mybir.AluOpType.mult, op1=mybir.AluOpType.mult)
            tpfull = pstp.tile([P, 1024], BF16, tag="tp", name="tpfull")
            tp = tpfull[:, 0:256]
            for cc in range(2):
                nc.tensor.transpose(tp[:, ts(cc, P)], nt[:, ts(cc, P)],
                                    identb)
            catv = cat[b][:].rearrange("p cc (r w) -> p cc r w", w=WPAD)
            nc.vector.tensor_tensor(
                out=catv[:, :, ds(2 * nch, 2), ds(1, 64)],
                in0=tp.rearrange("p (cc r w) -> p cc r w", cc=2, w=64),
                in1=xr[b][:, :, ds(nch * P, P)].rearrange(
                    "p cc (r w) -> p cc r w", w=64),
                op=mybir.AluOpType.add)
        return stage2

    for b in (1, 2):
        if b == 1:
            # branch-2 projections fill branch-1's ACT-bound g-slots
            conv_state["q"] = (
                list(vt_pieces(1))
                + list(proj_pieces(2, deferred=True))
                + list(vt_pieces(2)))
        else:
            conv_state["q"] = list(conv_half1_pieces())
        epi2 = []

        def av_burst(prev):
            # previous block's AV matmuls + psum-freeing stage-1 reads in
            # one PE burst ahead of the next block's ACT-paced scores;
            # stage-2 (nt/transpose/cat) is deferred into later g-slots.
            pex, pn0, pnw = prev
            for g in range(8):
                av = av_slice(b, pex, pn0, pnw, g)
                if av is not None:
                    sub = 0 if (pnw == P or g < 4) else 1
                    epi2.append(epilogue(b, pn0, sub, av))

        prev = None
        for j, (n0, nw) in enumerate(NB):
            ex = exp_pool.tile([P, 32, 256], F8, tag="ex", name=f"ex{b}_{j}")
            for g in range(8):
                ps = psc.tile([P, 1024], F32, tag="sc", name=f"sc{b}_{j}_{g}")
                for u in range(4):
                    mi = 4 * g + u
                    mm(ps[:, ds(u * nw, nw)], xk[b][:, :, ts(mi, P)],
                       qk[b][:, :, ds(n0, nw)],
                       start=True, stop=True, perf_mode=DR)
                nc.scalar.activation(
                    ex[:, ds(4 * g, 4), 0:nw], ps[:, 0:4 * nw],
                    EXP, bias=ebias, scale=0.125)
                if prev is not None:
                    pex, pn0, pnw = prev
                    av = av_slice(b, pex, pn0, pnw, g)
                    if av is not None:
                        sub = 0 if (pnw == P or g < 4) else 1
                        epi2.append(epilogue(b, pn0, sub, av))
                if epi2 and len(epi2) > 1:
                    epi2.pop(0)()
                emit_conv_piece()
            prev = (ex, n0, nw)
        # drain the final block's AV + epilogue
        pex, pn0, pnw = prev
        for g in range(8):
            av = av_slice(b, pex, pn0, pnw, g)
            if av is not None:
                sub = 0 if (pnw == P or g < 4) else 1
                epi2.append(epilogue(b, pn0, sub, av))
            while len(epi2) > 1:
                epi2.pop(0)()
            emit_conv_piece()
        while epi2:
            epi2.pop(0)()
        for _ in range(4):
            emit_conv_piece()
        # branch output
        ov = cat[b][:].rearrange("p cc (r w) -> p cc r w", w=WPAD)
        nc.sync.dma_start(out=t[f"o{b}"][:], in_=ov[:, :, ds(1, 32), :])
    while conv_state["q"]:
        emit_conv_piece()

    # ---- conv half-2 (cat2) + merge + BN/ReLU ----
    for win_i in range(5):
        ws, ww = FWINS[win_i]
        for oc in range(2):
            pool = psc if win_i % 2 == 0 else psconv
            pst = pool.tile([P, 512] if pool is psconv else [P, 1024],
                            F32, tag="cps" if pool is psconv else "sc",
                            name=f"c2_{win_i}_{oc}")
            conv_state["ps"] = pst
            for icc_rel in range(2):
                conv_half_quantum(2, win_i, oc, icc_rel)
            fp = fprep.tile([P, 512], BF16, tag="fp")
            nc.vector.tensor_tensor(out=fp[:, 0:ww], in0=pst[:, 0:ww],
                                    in1=convsb[:, oc, ds(ws, ww)],
                                    op=mybir.AluOpType.add)
            nc.scalar.activation(featv[:, oc, ds(ws, ww)], fp[:, 0:ww],
                                 RELU, bias=cbeta_sb[:, ds(oc, 1)],
                                 scale=cinv_sb[:, ds(oc, 1)])
    nc.sync.dma_start(out=t["feat"][:], in_=featv[:])


def _build():
    if "nc" in _CACHE:
        return _CACHE["nc"]
    nc = bacc.Bacc(None, target_bir_lowering=False)
    t = _declare_io(nc)
    from contextlib import ExitStack
    with tile.TileContext(nc) as tc, ExitStack() as ctx:
        _emit(nc, tc, t, ctx)
    nc.finalize()
    _CACHE["nc"] = nc
    return nc


def _prep_host(inputs):
    import ml_dtypes
    F8NP = ml_dtypes.float8_e4m3
    BFNP = ml_dtypes.bfloat16
    d = {k: np.ascontiguousarray(np.asarray(v, np.float32))
         for k, v in inputs.items()}
    gamma = float(d["gamma"].reshape(-1)[0])
    inv = d["bn_scale"] / np.sqrt(d["bn_var"] + EPS)
    beta = d["bn_bias"] - d["bn_mean"] * inv

    def f8(x):
        return np.clip(x, -200.0, 200.0).astype(F8NP)

    shared = {
        "cinv": np.ascontiguousarray(inv.reshape(2, P).T),
        "cbeta": np.ascontiguousarray(beta.reshape(2, P).T),
        "identb": np.eye(P, dtype=BFNP),
    }
    bq_cat = np.concatenate([d["bq1"], d["bq2"]])  # [64]
    vq = np.zeros((P, 2, 2), np.float32)
    for b in (1, 2):
        wkb = d[f"wk{b}"]      # [64, 256]
        wvb = d[f"wv{b}"]      # [256, 256]
        for i in (1, 2):
            wqi = d[f"wq{i}"]  # [32, 256]
            mm_bi = wqi.T @ wkb[32 * (i - 1):32 * i, :]  # [256 c', 256 c]
            shared[f"wm{b}{i}"] = f8(
                (256.0 * mm_bi).reshape(2, P, C).transpose(1, 0, 2))
        vqb = wkb.T @ bq_cat  # [256]
        vq[:, :, b - 1] = (8.0 * vqb).reshape(2, P).T
        # wv: [p, kc, co] = 16*wv[co, kc*128+p]
        shared[f"wv{b}"] = f8(
            (SW * wvb.T).reshape(2, P, C).transpose(1, 0, 2))
    shared["vq"] = vq
    # wct: [ic_p, icc, tap, occ, oc] = w_cat[occ*128+oc, icc*128+ic_p, ky, kx]
    wcat = d["w_cat"]  # [256, 512, 3, 3]
    shared["wct"] = np.ascontiguousarray(
        wcat.reshape(2, P, 4, P, 9).transpose(3, 2, 4, 0, 1)).astype(BFNP)

    gbv = {1: gamma * d["bv1"], 2: gamma * d["bv2"]}
    in_maps = []
    for core in range(8):
        s, half = core // 2, core % 2
        h0 = 32 * half
        shift = (h0 - 1) * 64
        m = dict(shared)
        fake_row = 0 if half == 0 else 33
        maskg = np.full((34, 64), gamma, np.float32)
        maskg[fake_row] = 0.0
        m["maskg"] = np.ascontiguousarray(
            maskg.reshape(NCH, P).T)
        for b in (1, 2):
            x = d[f"input{b}"][s].reshape(C, M)
            x_rot = np.roll(x, -shift, axis=1)
            m[f"xk{b}"] = f8(x_rot.reshape(2, P, M).transpose(1, 0, 2))
            xw = x_rot[:, :NQ] + gbv[b][:, None]
            xw = xw.reshape(C, 34, 64).copy()
            xw[:, fake_row, :] = 0.0
            m[f"xr{b}"] = np.ascontiguousarray(
                xw.reshape(2, P, NQ).transpose(1, 0, 2)).astype(BFNP)
        in_maps.append(m)
    return in_maps


def _run_cached_pjrt(nc, in_maps):
    import jax
    import numpy as _np
    from jax.sharding import Mesh, PartitionSpec
    from jax.experimental.shard_map import shard_map
    from concourse import bass2jax, mybir as _mb

    n_cores = len(in_maps)
    if "pjrt" not in _CACHE:
        bass2jax.install_neuronx_cc_hook()
        in_names, out_names, out_avals, zero_shapes = [], [], [], []
        for alloc in nc.m.functions[0].allocations:
            if not isinstance(alloc, _mb.MemoryLocationSet):
                continue
            name = alloc.memorylocations[0].name
            if alloc.kind == "ExternalInput":
                if nc.partition_id_tensor is None or \
                        name != nc.partition_id_tensor.name:
                    in_names.append(name)
            elif alloc.kind == "ExternalOutput":
                out_names.append(name)
                shape = tuple(alloc.tensor_shape)
                dtype = _mb.dt.np(alloc.dtype)
                out_avals.append(jax.core.ShapedArray(shape, dtype))
                zero_shapes.append((shape, dtype))
        n_params = len(in_names)
        all_names = in_names + out_names
        pid_name = nc.partition_id_tensor.name if nc.partition_id_tensor else None
        if pid_name is not None:
            all_names = all_names + [pid_name]

        def _body(*args):
            operands = list(args)
            if pid_name is not None:
                operands.append(bass2jax.partition_id_tensor())
            outs = bass2jax._bass_exec_p.bind(
                *operands,
                out_avals=tuple(out_avals),
                in_names=tuple(all_names),
                out_names=tuple(out_names),
                lowering_input_output_aliases=(),
                sim_require_finite=True,
                sim_require_nnan=True,
                nc=nc,
            )
            return tuple(outs)

        devices = jax.devices()[:n_cores]
        mesh = Mesh(_np.asarray(devices), ("core",))
        n_outs = len(out_names)
        sharded = jax.jit(
            shard_map(_body, mesh=mesh,
                      in_specs=(PartitionSpec("core"),) * (n_params + n_outs),
                      out_specs=(PartitionSpec("core"),) * n_outs,
                      check_rep=False),
            donate_argnums=tuple(range(n_params, n_params + n_outs)),
            keep_unused=True,
        )
        _CACHE["pjrt"] = (sharded, in_names, out_names, out_avals, zero_shapes)

    sharded, in_names, out_names, out_avals, zero_shapes = _CACHE["pjrt"]
    n_cores_ax = len(in_maps)
    concat_in = [
        _np.concatenate([_np.asarray(in_maps[c][nm]) for c in range(n_cores_ax)], axis=0)
        for nm in in_names
    ]
    concat_zeros = [
        _np.zeros((n_cores_ax * s[0], *s[1:]), d) for s, d in zero_shapes
    ]
    out_arrs = sharded(*concat_in, *concat_zeros)
    return [
        {nm: _np.asarray(out_arrs[i]).reshape(n_cores_ax, *out_avals[i].shape)[c]
         for i, nm in enumerate(out_names)}
        for c in range(n_cores_ax)
    ]


def kernel(**inputs):
    nc = _build()
    in_maps = _prep_host(inputs)
    try:
        results = _run_cached_pjrt(nc, in_maps)
    except Exception:
        _CACHE.pop("pjrt", None)
        res = run_bass_kernel_spmd(nc, in_maps, core_ids=list(range(8)))
        _CACHE["last_results"] = res
        results = res.results
    feat = np.zeros((4, C, 64, 64), np.float32)
    o1 = np.zeros((4, C, 64, 64), np.float32)
    o2 = np.zeros((4, C, 64, 64), np.float32)
    for core in range(8):
        s, half = core // 2, core % 2
        r = results[core]
        sl = slice(32 * half, 32 * half + 32)
        # o outputs: [P, 2, 32, 66] bf16, cols 1..64 are data
        for name, dst in (("o1", o1), ("o2", o2)):
            v = np.asarray(r[name]).astype(np.float32)[:, :, :, 1:65]
            dst[s, :, sl] = v.transpose(1, 0, 2, 3).reshape(C, 32, 64)
        fv = np.asarray(r["feat"]).astype(np.float32)  # [P, 2, 2110]
        fimg = np.zeros((P, 2, 32, 64), np.float32)
        flat = fv.reshape(P, 2, FEATW)
        for rr in range(32):
            fimg[:, :, rr, :] = flat[:, :, rr * 66:rr * 66 + 64]
        feat[s, :, sl] = fimg.transpose(1, 0, 2, 3).reshape(C, 32, 64)
    return (feat, o1, o2)
